# revision 46
# baseline (speedup 1.0000x reference)
"""Trainium2 Bass kernel for nn_Attention_37074157699274.

Multi-head self-attention over tiny 32-token groups:
  x [4, 1024, 32, 256] -> qkv -> per-(b,p)-group 8-head attention -> out proj.

Strategy: data-parallel over the 4096 (b,p) groups across 8 NeuronCores
(512 groups/core).  On-core, groups are processed in blocks of 4 (=128
tokens, one partition span), in a three-stage software-skewed pipeline:
round i runs block i's QKV, block i-1's dots (whose exp then has a full
round of ACT slack), and block i-2's post-softmax work, so the in-order
PE queue never waits on a same-round exp.  Per block:
  - x loaded by SWDGE with an in-flight f32->bf16 cast; xT via two bf16
    PE transposes (bitcast-packed into PSUM), evacuated on DVE.
  - QKV projection on PE: q,k feature-major (heads land at partition
    offsets usable as matmul tiles), v token-major (18 matmuls).
  - dotsT via 32 tiny [32x32] k_g^T q_g matmuls packed with tile_position
    (lhsT/rhs swapped vs the naive dots: the output IS the transposed
    attention matrix, so no attention transpose is ever needed).
  - exp on ACT (fused *0.125 scale) -> compact bf16 [128, 8*32].
  - softmax denominators via 32 N=1 ones-column matmuls (virtually free
    on PE, replacing a DVE segmented reduce).
  - attn@v as 32 tiny matmuls on the unnormalized exp; normalization
    (reciprocal + broadcast multiply) fused into the DVE evacuation.
  - oT via 4 bf16 PE transposes; out projection consumes oT chunks as
    stationary operands; bias added during the PSUM evacuation on DVE;
    SWDGE store per block pair.
PSUM (8/8 banks): attn pool [128,1024] bufs=3 carries v -> dotsT -> exp
read + sums + o' + oT staging + out-projection per block; qkv pool
[128,1024] bufs=1 (its evacuations clear with most of a round to spare).
The preamble is ordered for the earliest possible block-0 start
(identities first, W_qkv loaded and cast in dc-halves).
"""

import numpy as np

import concourse.bacc as bacc
import concourse.bass as bass
from concourse import bass_utils, mybir
from concourse.tile import TileContext

F32 = mybir.dt.float32
BF16 = mybir.dt.bfloat16
AF = mybir.ActivationFunctionType
ALU = mybir.AluOpType
AX = mybir.AxisListType

B, P, N, DIM = 4, 1024, 32, 256
HEADS, DH, INNER = 8, 64, 512
SCALE = DH ** -0.5
NCORES = 8
GROUPS = B * P                   # 4096 independent attention groups
GPC = GROUPS // NCORES           # 512 groups per core
BLK = 128                        # tokens per block = 4 groups
GPB = BLK // N                   # 4 groups per block


def build_kernel_body(tc, x_d, wqkv_d, wout_d, bout_d, out_d, nblk):
    nc = tc.nc

    # ---------------- one-time weight prep ----------------
    # Ordered so block 0 starts as early as possible: identities first
    # (the first xT transpose needs only ident_b + the x pair), then W_qkv
    # loaded and cast in dc-halves so the dc=0 qkv matmuls do not wait for
    # the whole weight tensor; W_out/bias are not needed until rounds later.
    with tc.tile_pool(name="wpool", bufs=1) as wp:
        from concourse.masks import make_identity
        ident_f = wp.tile([128, 128], F32, name="ident_f")
        make_identity(nc, ident_f)
        ident_b = wp.tile([128, 128], BF16, name="ident_b")
        make_identity(nc, ident_b)

        ones_c = wp.tile([128, 1], BF16, name="ones_c")
        nc.vector.memset(ones_c, 1.0)

        # W_qkv [256, 1536] -> [128 part, dchunk 2, 1536] bf16, per half
        wqkv_v = wqkv_d.rearrange("(c p) f -> p c f", c=2)
        wqkv_f = wp.tile([128, 2, 3 * INNER], F32, name="wqkv_f")
        wqkv_b = wp.tile([128, 2, 3 * INNER], BF16, name="wqkv_b")
        for dc in range(2):
            nc.sync.dma_start(out=wqkv_f[:, dc], in_=wqkv_v[:, dc])
            nc.vector.tensor_copy(wqkv_b[:, dc], wqkv_f[:, dc])

        # W_out [512, 256] -> [128 part, chunk 4, 256] bf16
        wout_f = wp.tile([128, 4, DIM], F32, name="wout_f")
        nc.sync.dma_start(out=wout_f, in_=wout_d.rearrange("(c p) f -> p c f", c=4))
        wout_b = wp.tile([128, 4, DIM], BF16, name="wout_b")
        nc.vector.tensor_copy(wout_b, wout_f)

        # bias replicated across partitions [128, 256] f32
        bias_t = wp.tile([128, DIM], F32, name="bias_t")
        nc.sync.dma_start(out=bias_t, in_=bout_d.unsqueeze(0).broadcast_to([128, DIM]))

        _main_loop(tc, x_d, out_d, nblk, wqkv_b, wout_b, bias_t,
                   ident_f, ident_b, ones_c)


def _main_loop(tc, x_d, out_d, nblk, wqkv_b, wout_b, bias_t,
               ident_f, ident_b, ones_c):
    nc = tc.nc
    assert nblk % 2 == 0
    # x viewed as [pair, token-in-block 128, block-in-pair 2, 256]
    xv2 = x_d.rearrange("(n b p) d -> n p b d", b=2, p=BLK)
    ov2 = out_d.rearrange("(n b p) d -> n p b d", b=2, p=BLK)

    with (
        tc.tile_pool(name="io", bufs=4) as iop,
        tc.tile_pool(name="work", bufs=4) as wk,
        tc.tile_pool(name="ps_qkv", bufs=1, space="PSUM") as pqkv,
        tc.tile_pool(name="ps_attn", bufs=3, space="PSUM") as pat,
    ):
        state = {}

        def stage_a(i):
            # ---- load x (one SWDGE DMA per 2 blocks, f32->bf16 cast
            # in-flight on the software DGE) ----
            if i % 2 == 0:
                state["x_f2"] = iop.tile([128, 2, DIM], BF16, tag="x_f2",
                                         name="x_f2")
                nc.gpsimd.dma_start(out=state["x_f2"], in_=xv2[i // 2])
            x_f2 = state["x_f2"]

            qk_ps = pqkv.tile([128, 1024], F32, tag="qk_ps", name="qk_ps")
            attn_ps = pat.tile([128, 1024], F32, tag="attn_ps", name="attn_ps")

            # ---- transpose x via PE (bf16, bitcast-packed); DVE evacuates
            for dc in range(2):
                nc.tensor.transpose(
                    qk_ps[:, 64 * dc:64 * dc + 64].bitcast(BF16),
                    x_f2[:, i % 2, 128 * dc:128 * dc + 128], ident_b)
            xT = wk.tile([128, 2, 128], BF16, tag="xT", name="xT")
            nc.vector.tensor_copy(
                xT.rearrange("p a b -> p (a b)"),
                qk_ps[:, 0:128].bitcast(BF16))

            # ---- qkv projection ----
            # q,k feature-major into a 2-bank tile; v token-major goes into
            # bank 1 of attn_ps (its evac precedes any dots write there).
            for c in range(8):
                for dc in range(2):
                    nc.tensor.matmul(
                        qk_ps[:, 128 * c:128 * c + 128],
                        lhsT=wqkv_b[:, dc, 128 * c:128 * c + 128],
                        rhs=xT[:, dc],
                        start=(dc == 0), stop=(dc == 1))
            for dc in range(2):
                nc.tensor.matmul(
                    attn_ps[:, 512:1024],
                    lhsT=xT[:, dc],
                    rhs=wqkv_b[:, dc, 2 * INNER:3 * INNER],
                    start=(dc == 0), stop=(dc == 1))

            # split evacuation: ACT takes q then v, DVE takes k (parallel)
            qkv_sb = wk.tile([128, 1536], BF16, tag="qkv_sb", name="qkv_sb")
            nc.scalar.copy(qkv_sb[:, 0:512], qk_ps[:, 0:512])
            nc.vector.tensor_copy(qkv_sb[:, 512:1024], qk_ps[:, 512:1024])
            nc.scalar.copy(qkv_sb[:, 1024:1536], attn_ps[:, 512:1024])
            return attn_ps, qkv_sb

        def stage_b1(i, attn_ps, qkv_sb):
            q_sb = qkv_sb[:, 0:512]
            k_sb = qkv_sb[:, 512:1024]

            # ---- dotsT: per (group g, head h) 32x32 = k_g^T q_g, packed via
            # tile_position.  Same mechanics as the original dots, but with
            # lhsT/rhs swapped the output IS the transposed attention matrix,
            # so no 32x32 DVE stream-transpose is ever needed.
            for h in range(HEADS):
                c, pp = h // 2, h % 2
                for g in range(GPB):
                    col = 128 * c + 32 * g
                    dcol = 512 * pp + 32 * c
                    nc.tensor.matmul(
                        attn_ps[32 * g:32 * g + 32, dcol:dcol + 32],
                        lhsT=k_sb[64 * pp:64 * pp + 64, col:col + 32],
                        rhs=q_sb[64 * pp:64 * pp + 64, col:col + 32],
                        start=True, stop=True,
                        tile_position=(64 * pp, 32 * g))

            # ---- exp (fused 1/8 scale); layout [(g,j), (h-col, i)] ----
            em = wk.tile([128, 256], BF16, tag="em", name="em")
            dots_view = attn_ps.rearrange(
                "p (b x) -> p b x", b=2)[:, :, 0:128]
            nc.scalar.activation(
                em.rearrange("p (b x) -> p b x", b=2),
                dots_view, AF.Exp, bias=0.0, scale=SCALE)

            return em

        def stage_b2(i, attn_ps, qkv_sb, em):
            v_sb = qkv_sb[:, 1024:1536]
            # ---- softmax denominators: per (g,h) an N=1 matmul against a
            # ones column sums expT over j; lands token-major in bank 1
            # (cols 512+: free after the exp read of the pp=1 dots).
            for h in range(HEADS):
                c, pp = h // 2, h % 2
                acol = 128 * pp + 32 * c
                for g in range(GPB):
                    nc.tensor.matmul(
                        attn_ps[32 * g:32 * g + 32, 640 + h:641 + h],
                        lhsT=em[32 * g:32 * g + 32, acol:acol + 32],
                        rhs=ones_c[32 * g:32 * g + 32],
                        start=True, stop=True,
                        tile_position=(32 * g, 32 * g))

            # ---- attn @ v -> unnormalized o (token-major) ----
            o_ps = attn_ps[:, 0:512]
            for h in range(HEADS):
                c, pp = h // 2, h % 2
                acol = 128 * pp + 32 * c
                for g in range(GPB):
                    nc.tensor.matmul(
                        o_ps[32 * g:32 * g + 32, 64 * h:64 * h + 64],
                        lhsT=em[32 * g:32 * g + 32, acol:acol + 32],
                        rhs=v_sb[32 * g:32 * g + 32, 64 * h:64 * h + 64],
                        start=True, stop=True,
                        tile_position=(32 * g, 32 * g))

            # ---- normalize during evacuation: o_sb = o' * (1/s) ----
            r_t = wk.tile([128, 8], F32, tag="r_t", name="r_t")
            nc.vector.reciprocal(r_t, attn_ps[:, 640:648])
            o_sb = wk.tile([128, 512], BF16, tag="o_sb", name="o_sb")
            nc.vector.tensor_mul(
                o_sb.rearrange("p (h e) -> p h e", h=HEADS),
                o_ps.rearrange("p (h e) -> p h e", h=HEADS),
                r_t.unsqueeze(2).broadcast_to([128, 8, 64]))
            # transpose o to inner-major via PE (4x 128x128), reusing bank 0
            # (bf16 pairs packed into fp32 PSUM cells via bitcast views)
            for c in range(4):
                nc.tensor.transpose(
                    attn_ps[:, 64 * c:64 * c + 64].bitcast(BF16),
                    o_sb[:, 128 * c:128 * c + 128], ident_b)
            oT_sb = wk.tile([128, 4, 128], BF16, tag="oT_sb", name="oT_sb")
            nc.vector.tensor_copy(
                oT_sb.rearrange("p a b -> p (a b)"),
                attn_ps[:, 0:256].bitcast(BF16))

            # ---- out projection: accumulate over 4 inner chunks ----
            op_ps = attn_ps[:, 512:768]
            for c in range(4):
                nc.tensor.matmul(
                    op_ps,
                    lhsT=oT_sb[:, c],
                    rhs=wout_b[:, c],
                    start=(c == 0), stop=(c == 3))

            if i % 2 == 0:
                state["out_sb2"] = iop.tile([128, 2, DIM], F32, tag="out_sb2",
                                            name="out_sb2")
            nc.vector.scalar_tensor_tensor(
                out=state["out_sb2"][:, i % 2], in0=op_ps, scalar=1.0,
                in1=bias_t, op0=ALU.mult, op1=ALU.add)
            if i % 2 == 1:
                nc.gpsimd.dma_start(out=ov2[i // 2], in_=state["out_sb2"])

        # software-skewed emission, three stages deep: round i runs block
        # i's qkv, block i-1's dotsT (whose exp then has a full round on ACT
        # before anyone needs it), and block i-2's post-softmax work, so the
        # in-order PE queue never waits on a same-round exp.
        stages = {}
        for i in range(nblk + 2):
            if i < nblk:
                stages[i] = list(stage_a(i))
            if 1 <= i <= nblk:
                stages[i - 1].append(stage_b1(i - 1, *stages[i - 1]))
            if i >= 2:
                stage_b2(i - 2, *stages.pop(i - 2))


def build(nblk):
    nc = bacc.Bacc("TRN2", target_bir_lowering=False, debug=False,
                   enable_asserts=False)
    tok = nblk * BLK
    x_d = nc.dram_tensor("x", [tok, DIM], F32, kind="ExternalInput").ap()
    wqkv_d = nc.dram_tensor("w_qkv", [DIM, 3 * INNER], F32,
                            kind="ExternalInput").ap()
    wout_d = nc.dram_tensor("w_out", [INNER, DIM], F32,
                            kind="ExternalInput").ap()
    bout_d = nc.dram_tensor("b_out", [DIM], F32, kind="ExternalInput").ap()
    out_d = nc.dram_tensor("out", [tok, DIM], F32, kind="ExternalOutput").ap()
    with TileContext(nc) as tc:
        build_kernel_body(tc, x_d, wqkv_d, wout_d, bout_d, out_d, nblk)
    nc.compile()
    return nc


_NC_CACHE = {}


def _get_nc(nblk):
    if nblk not in _NC_CACHE:
        _NC_CACHE[nblk] = build(nblk)
    return _NC_CACHE[nblk]


def kernel(x, W_qkv, W_out, b_out, trace=False):
    assert x.shape == (B, P, N, DIM)
    nblk = GPC * N // BLK        # 128 blocks/core
    nc = _get_nc(nblk)
    xf = np.ascontiguousarray(x.reshape(GROUPS * N, DIM).astype(np.float32))
    shards = xf.reshape(NCORES, GPC * N, DIM)
    in_maps = [
        {"x": shards[i], "w_qkv": np.asarray(W_qkv, np.float32),
         "w_out": np.asarray(W_out, np.float32),
         "b_out": np.asarray(b_out, np.float32)}
        for i in range(NCORES)
    ]
    res = bass_utils.run_bass_kernel_spmd(
        nc, in_maps, core_ids=list(range(NCORES)), trace=trace)
    out = np.concatenate([res.results[i]["out"] for i in range(NCORES)], axis=0)
    out = out.reshape(B, P, N, DIM).astype(np.float32)
    if trace:
        return out, res
    return out

